# revision 1
# baseline (speedup 1.0000x reference)
"""AttentivePool Trainium2 kernel.

Reference computation (per batch sample b):
    m[c, w]   = mean_h x[b, c, h, w]                      # H-mean pool
    s[c', w]  = tanh(sum_c W[c, c'] m[c, w] + bias[c'])   # additive attention
    a[w]      = sum_c' s[c', w] proj[c']
    p[w]      = softmax_w(a)                              # over W
    out[b, c] = sum_w p[w] m[c, w]

Strategy: pure data-parallel over B across 8 cores (2 samples/core).
x is 1 GiB; everything else is tiny -> memory-bound on streaming x
(measured ~260 GB/s/core with all 8 cores streaming -> ~516 us floor).
Per 256-wide W-chunk: one strided DVE reduce computes the H-sum (view
the tile as [c, w, h], reduce innermost; fp32 PE matmuls run at 1/4
rate so a matmul-based mean would exceed the DMA roofline), small PE
matmuls handle the two projections and the partition-broadcast of the
softmax numerator, ACT does tanh/exp (exp also emits its chunk-sum via
accum_out for the softmax denominator), and DVE mul+reduce forms the
weighted partial sums. The attention tail of chunk k is emitted after
chunk k+1's big reduce (software pipeline) so the in-order DVE queue
never stalls on the tail's 5-hop cross-engine chain. x loads alternate
between the two HWDGE rings (SP and ACT). Softmax skips the
max-subtraction: |a| <= sum|proj| = 12.8 hard bound, exp is safe in f32.

Scaling: the reduce computes H*mean; the host folds 1/H into weight_W
and into the broadcast ones-vector so all downstream values come out
exact (1/H is a power of two).
"""

import contextlib

import numpy as np

import concourse.bacc as bacc
import concourse.tile as tile
from concourse import mybir
from concourse.bass_utils import run_bass_kernel_spmd

B, C, H, W = 16, 128, 32, 4096
N_CORES = 8
BL = B // N_CORES  # batch samples per core
WT = 256           # W-chunk width
F32 = mybir.dt.float32


def build_bass(bl=BL, w=W, wt=WT, reps=1, loop_reps=1, xbufs=5, dual_q=True,
               dma_scratch=16384, mbufs=5, tbufs=2, pbufs=(2, 2, 2)):
    nch = w // wt
    # Bacc (not plain Bass): its compile() runs generate_event_semaphores,
    # which spills >1-wait sync conditions into EventSemaphore instructions
    # (the TRN2 ISA allows a single wait slot per instruction).
    nc = bacc.Bacc(trn_type="TRN2", dynamic_dma_scratch_size=dma_scratch)

    x = nc.dram_tensor("x", [bl, C, H, w], F32, kind="ExternalInput")
    # All small parameters packed into one tensor: a single DMA means every
    # PE matmul depends on a single weight semaphore (the Matmult/LdWeights
    # sync slot only fits ONE wait, so fan-in must stay at 1).
    # cols 0:128 = weight_W/H, 128:256 = eye(C), 256 = proj, 257 = bias,
    # row 0 of cols 258:386 = 1/H (ones row for broadcasts).
    wpack = nc.dram_tensor("wpack", [C, 386], F32, kind="ExternalInput")
    out = nc.dram_tensor("out", [bl, C], F32, kind="ExternalOutput")

    with tile.TileContext(nc) as tc:
        with (
            tc.tile_pool(name="singles", bufs=1) as singles,
            tc.tile_pool(name="xp", bufs=xbufs) as xpool,
            tc.tile_pool(name="mp", bufs=mbufs) as mpool,
            tc.tile_pool(name="sqp", bufs=tbufs) as sqpool,
            tc.tile_pool(name="ep", bufs=tbufs) as epool,
            tc.tile_pool(name="accp", bufs=1) as accp,
            tc.tile_pool(name="psp", bufs=pbufs[0], space="PSUM") as psp,
            tc.tile_pool(name="pbp", bufs=pbufs[1], space="PSUM") as pbp,
            tc.tile_pool(name="psmall", bufs=pbufs[2], space="PSUM") as psmall,
        ):
            # wpack rides the ACT ring so x-chunk 0 (SP ring) starts at t=0.
            sb_w = singles.tile([C, 386], F32, tag="wpack")
            nc.scalar.dma_start(out=sb_w, in_=wpack[:, :])
            sb_ww = sb_w[:, 0:C]
            sb_ident = sb_w[:, C:2 * C]
            sb_proj = sb_w[:, 2 * C:2 * C + 1]
            sb_bias = sb_w[:, 2 * C + 1:2 * C + 2]
            sb_ones = sb_w[0:1, 2 * C + 2:3 * C + 2]

            # Dummy matmul so PE observes the wpack DMA semaphore before the
            # main loop; later matmuls then only wait on their data operand.
            scr = psmall.tile([1, 1], F32, tag="small")
            nc.tensor.matmul(scr, sb_proj, sb_bias, start=True, stop=True)

            # Per-(b, chunk) accumulators, each slot written exactly once.
            partials = accp.tile([C, bl, nch], F32, tag="partials")
            dparts = accp.tile([1, bl, nch], F32, tag="dparts")

            def tail_head(b, k, mt):
                # squish = tanh((W/H)^T (H*mean) + bias)
                pst = psp.tile([C, wt], F32, tag="ps", name="pst")
                nc.tensor.matmul(pst, sb_ww, mt, start=True, stop=True)
                sqt = sqpool.tile([C, wt], F32, tag="sq", name="sqt")
                nc.scalar.activation(
                    out=sqt, in_=pst,
                    func=mybir.ActivationFunctionType.Tanh,
                    bias=sb_bias, scale=1.0,
                )
                # attn chunk = proj^T squish  -> [1, wt]
                pat = psmall.tile([1, wt], F32, tag="small", name="pat")
                nc.tensor.matmul(pat, sb_proj, sqt, start=True, stop=True)
                # exp (softmax numerator); accum_out = chunk sum for denom
                et = epool.tile([1, wt], F32, tag="et", name="et")
                nc.scalar.activation(
                    out=et, in_=pat,
                    func=mybir.ActivationFunctionType.Exp,
                    accum_out=dparts[0:1, b, k:k + 1],
                )
                return et

            def tail_rest(b, k, mt, et):
                # broadcast exp/H to all partitions, then
                # (exp/H)*(H*mean) summed over w -> partials[:, b, k].
                # (tensor_tensor_reduce hard-faults TRN2, so separate
                # mul — in-place over the dead mt — plus reduce.)
                pbt = pbp.tile([C, wt], F32, tag="pb", name="pbt")
                nc.tensor.matmul(pbt, sb_ones, et, start=True, stop=True)
                nc.vector.tensor_mul(out=mt, in0=pbt, in1=mt)
                nc.vector.reduce_sum(
                    out=partials[:, b, k:k + 1], in_=mt,
                    axis=mybir.AxisListType.X,
                )

            def attention_tail(b, k, mt):
                et = tail_head(b, k, mt)
                tail_rest(b, k, mt, et)

            loop_cm = (
                tc.For_i(0, loop_reps, 1) if loop_reps > 1
                else contextlib.nullcontext()
            )
            with loop_cm:
              for _rep in range(reps):
                # Software pipeline: chunk k's attention tail is emitted
                # after chunk k+1's big reduce, so the in-order DVE queue
                # never stalls on the PE/ACT round-trips of the tail (the
                # tail's mul waits on a 5-hop cross-engine chain).
                pending = None
                for b in range(bl):
                    for k in range(nch):
                        ws = slice(k * wt, (k + 1) * wt)
                        xt = xpool.tile([C, H, wt], F32, tag="xt")
                        i = b * nch + k
                        dma_eng = nc.scalar if (dual_q and i % 2) else nc.sync
                        dma_eng.dma_start(out=xt, in_=x[b, :, :, ws])
                        # H*mean[c, ws]: one strided DVE reduce — view the
                        # tile as [c, w, h] so h is the innermost (X) axis.
                        # (fp32 matmuls run at 1/4 rate, so a PE identity-
                        # matmul chain here would exceed the DMA roofline.)
                        mt = mpool.tile([C, wt], F32, tag="mt")
                        nc.vector.reduce_sum(
                            out=mt, in_=xt.rearrange("c h w -> c w h"),
                            axis=mybir.AxisListType.X,
                        )
                        if pending is not None:
                            attention_tail(*pending)
                        pending = (b, k, mt)
                # Drain: after the final exp, dparts is complete, so the
                # denominator chain is emitted between the last tail's head
                # and rest — it runs in parallel with the weighted-sum tail
                # instead of serially after it.
                lb, lk, lmt = pending
                let = tail_head(lb, lk, lmt)
                drow = accp.tile([1, bl], F32, tag="drow")
                nc.vector.reduce_sum(
                    out=drow, in_=dparts, axis=mybir.AxisListType.X)
                nc.scalar.mul(out=drow, in_=drow, mul=1.0 / H)
                nc.vector.reciprocal(out=drow, in_=drow)  # H/denom
                pdb = psmall.tile([C, bl], F32, tag="pdb")
                # (1/H) ones^T @ (H/denom) = 1/denom bcast to all partitions
                nc.tensor.matmul(pdb, sb_ones, drow, start=True, stop=True)
                tail_rest(lb, lk, lmt, let)

            rescol = accp.tile([C, bl], F32, tag="rescol")
            nc.vector.reduce_sum(out=rescol, in_=partials, axis=mybir.AxisListType.X)
            resn = accp.tile([C, bl], F32, tag="resn")
            nc.vector.tensor_mul(out=resn, in0=rescol, in1=pdb)

            # out[b, c] = resn[c, b]: transpose via matmul with identity.
            pt = psmall.tile([bl, C], F32, tag="small")
            nc.tensor.matmul(pt, resn, sb_ident, start=True, stop=True)
            out_sb = accp.tile([bl, C], F32, tag="out_sb")
            nc.vector.tensor_copy(out=out_sb, in_=pt)
            nc.sync.dma_start(out=out[:, :], in_=out_sb)

    nc.compile()
    return nc


def make_in_maps(x, weight_W, weight_proj, bias, bl=BL, n_cores=N_CORES):
    x = np.ascontiguousarray(np.asarray(x, dtype=np.float32))
    wpack = np.zeros((C, 386), dtype=np.float32)
    wpack[:, 0:C] = np.asarray(weight_W, dtype=np.float32) / np.float32(H)
    wpack[:, C:2 * C] = np.eye(C, dtype=np.float32)
    wpack[:, 2 * C:2 * C + 1] = np.asarray(weight_proj, dtype=np.float32)
    wpack[:, 2 * C + 1:2 * C + 2] = np.asarray(bias, dtype=np.float32)
    wpack[0, 2 * C + 2:3 * C + 2] = 1.0 / np.float32(H)
    return [
        {
            "x": np.ascontiguousarray(x[i * bl:(i + 1) * bl]),
            "wpack": wpack,
        }
        for i in range(n_cores)
    ]


_NC_CACHE = {}


def kernel(x, weight_W, weight_proj, bias, **run_kwargs):
    if "nc" not in _NC_CACHE:
        _NC_CACHE["nc"] = build_bass()
    nc = _NC_CACHE["nc"]
    in_maps = make_in_maps(x, weight_W, weight_proj, bias)
    res = None
    for attempt in range(3):
        try:
            res = run_bass_kernel_spmd(
                nc, in_maps, core_ids=list(range(N_CORES)), **run_kwargs)
            break
        except Exception:
            # Transient NRT/device hiccups recover on retry; re-raise if not.
            if attempt == 2:
                raise
    out = np.concatenate([r["out"] for r in res.results], axis=0)
    if run_kwargs:
        kernel.last_results = res
    return out



# revision 2
# speedup vs baseline: 1.3248x; 1.3248x over previous
"""AttentivePool Trainium2 kernel.

Reference computation (per batch sample b):
    m[c, w]   = mean_h x[b, c, h, w]                      # H-mean pool
    s[c', w]  = tanh(sum_c W[c, c'] m[c, w] + bias[c'])   # additive attention
    a[w]      = sum_c' s[c', w] proj[c']
    p[w]      = softmax_w(a)                              # over W
    out[b, c] = sum_w p[w] m[c, w]

Strategy: pure data-parallel over B across 8 cores (2 samples/core).
x is 1 GiB; everything else is tiny -> memory-bound on streaming x.

DMA layout is the whole game: loading [C, H, wt] w-chunks (as a strided
reduce would want) gives only wt*4-byte contiguous HBM segments — 1 KiB
at wt=256 — and measures ~260 GB/s/core.  Loading h-slices x[b, :, h, :]
as [C, W] tiles gives 16 KiB contiguous segments per partition and
measures ~354 GB/s/core, at the ~358 GB/s HBM-per-NeuronCore limit
(716 GB/s/stack shared by 2 cores).  So: stream 32 h-slices per sample
(2 MiB DMAs alternating between the SP and ACT HWDGE rings) and form
the H-sum with 31 in-place DVE tensor_adds (fp32 tensor_tensor runs at
1 elem/cycle/lane -> 4.4 us per add, under the 5.9 us DMA inter-arrival,
so DVE hides completely under the DMA stream).

The attention tail runs per 512-wide chunk (one PSUM bank each): PE
matmul (W/H)^T m -> ACT tanh+bias -> PE proj matmul -> ACT exp (with
accum_out emitting the chunk-sum for the softmax denominator) -> PE
ones-broadcast -> DVE mul+reduce for the weighted partial sums.  Chunk
tails of sample b are emitted interleaved into sample b+1's add stream
(software pipeline) so the in-order DVE queue never stalls on the tail's
cross-engine latency; the last sample's tails drain at the end (~10 us).
Softmax skips the max-subtraction: |a| <= sum|proj| = 12.8 hard bound,
exp is safe in f32.

Scaling: the adds compute H*mean; the host folds 1/H into weight_W and
into the broadcast ones-vector so all downstream values come out exact
(1/H is a power of two).
"""

import contextlib

import numpy as np

import concourse.bacc as bacc
import concourse.tile as tile
from concourse import mybir
from concourse.bass_utils import run_bass_kernel_spmd

B, C, H, W = 16, 128, 32, 4096
N_CORES = 8
BL = B // N_CORES  # batch samples per core
WT = 512           # attention-tail chunk width (one 2 KiB PSUM bank)
F32 = mybir.dt.float32


def build_bass(bl=BL, w=W, wt=WT, reps=1, loop_reps=1, xbufs=5, dual_q=True,
               dma_scratch=16384, tbufs=2, pbufs=(2, 2, 2)):
    nch = w // wt
    # Bacc (not plain Bass): its compile() runs generate_event_semaphores,
    # which spills >1-wait sync conditions into EventSemaphore instructions
    # (the TRN2 ISA allows a single wait slot per instruction).
    nc = bacc.Bacc(trn_type="TRN2", dynamic_dma_scratch_size=dma_scratch)

    x = nc.dram_tensor("x", [bl, C, H, w], F32, kind="ExternalInput")
    # All small parameters packed into one tensor: a single DMA means every
    # PE matmul depends on a single weight semaphore (the Matmult/LdWeights
    # sync slot only fits ONE wait, so fan-in must stay at 1).
    # cols 0:128 = weight_W/H, 128:256 = eye(C), 256 = proj, 257 = bias,
    # row 0 of cols 258:386 = 1/H (ones row for broadcasts).
    wpack = nc.dram_tensor("wpack", [C, 386], F32, kind="ExternalInput")
    out = nc.dram_tensor("out", [bl, C], F32, kind="ExternalOutput")

    with tile.TileContext(nc) as tc:
        with (
            tc.tile_pool(name="singles", bufs=1) as singles,
            tc.tile_pool(name="xp", bufs=xbufs) as xpool,
            tc.tile_pool(name="mp", bufs=1) as mpool,
            tc.tile_pool(name="sqp", bufs=tbufs) as sqpool,
            tc.tile_pool(name="ep", bufs=tbufs) as epool,
            tc.tile_pool(name="accp", bufs=1) as accp,
            tc.tile_pool(name="psp", bufs=pbufs[0], space="PSUM") as psp,
            tc.tile_pool(name="pbp", bufs=pbufs[1], space="PSUM") as pbp,
            tc.tile_pool(name="psmall", bufs=pbufs[2], space="PSUM") as psmall,
        ):
            # wpack rides the ACT ring so x h-slice 0 (SP ring) starts at t=0.
            sb_w = singles.tile([C, 386], F32, tag="wpack")
            nc.scalar.dma_start(out=sb_w, in_=wpack[:, :])
            sb_ww = sb_w[:, 0:C]
            sb_ident = sb_w[:, C:2 * C]
            sb_proj = sb_w[:, 2 * C:2 * C + 1]
            sb_bias = sb_w[:, 2 * C + 1:2 * C + 2]
            sb_ones = sb_w[0:1, 2 * C + 2:3 * C + 2]

            # Dummy matmul so PE observes the wpack DMA semaphore before the
            # main loop; later matmuls then only wait on their data operand.
            scr = psmall.tile([1, 1], F32, tag="small")
            nc.tensor.matmul(scr, sb_proj, sb_bias, start=True, stop=True)

            # Per-(b, chunk) accumulators, each slot written exactly once.
            partials = accp.tile([C, bl, nch], F32, tag="partials")
            dparts = accp.tile([1, bl, nch], F32, tag="dparts")

            def tail_head(b, k, mt):
                # squish = tanh((W/H)^T (H*mean) + bias) on chunk k of m_b
                pst = psp.tile([C, wt], F32, tag="ps", name="pst")
                nc.tensor.matmul(pst, sb_ww, mt, start=True, stop=True)
                sqt = sqpool.tile([C, wt], F32, tag="sq", name="sqt")
                nc.scalar.activation(
                    out=sqt, in_=pst,
                    func=mybir.ActivationFunctionType.Tanh,
                    bias=sb_bias, scale=1.0,
                )
                # attn chunk = proj^T squish  -> [1, wt]
                pat = psmall.tile([1, wt], F32, tag="small", name="pat")
                nc.tensor.matmul(pat, sb_proj, sqt, start=True, stop=True)
                # exp (softmax numerator); accum_out = chunk sum for denom
                et = epool.tile([1, wt], F32, tag="et", name="et")
                nc.scalar.activation(
                    out=et, in_=pat,
                    func=mybir.ActivationFunctionType.Exp,
                    accum_out=dparts[0:1, b, k:k + 1],
                )
                return et

            def tail_rest(b, k, mt, et):
                # broadcast exp/H to all partitions, then
                # (exp/H)*(H*mean) summed over w -> partials[:, b, k].
                # (tensor_tensor_reduce hard-faults TRN2, so separate
                # mul — in-place over the dead m chunk — plus reduce.)
                pbt = pbp.tile([C, wt], F32, tag="pb", name="pbt")
                nc.tensor.matmul(pbt, sb_ones, et, start=True, stop=True)
                nc.vector.tensor_mul(out=mt, in0=pbt, in1=mt)
                nc.vector.reduce_sum(
                    out=partials[:, b, k:k + 1], in_=mt,
                    axis=mybir.AxisListType.X,
                )

            def attention_tail(b, k, mt):
                et = tail_head(b, k, mt)
                tail_rest(b, k, mt, et)

            loop_cm = (
                tc.For_i(0, loop_reps, 1) if loop_reps > 1
                else contextlib.nullcontext()
            )
            with loop_cm:
              for _rep in range(reps):
                # Pipeline: sample b's chunk tails are emitted interleaved
                # into sample b+1's add stream (one tail per two h-slices)
                # so the in-order DVE queue never stalls on the tail's
                # cross-engine PE/ACT round trips.
                pending = []  # (b, k, m-chunk) tail work not yet emitted
                ms = []
                for b in range(bl):
                    m = mpool.tile([C, w], F32, tag=f"m{b}")
                    ms.append(m)
                    for h in range(H):
                        i = b * H + h
                        dma_eng = nc.scalar if (dual_q and i % 2) else nc.sync
                        if h == 0:
                            # First h-slice lands directly in the m
                            # accumulator; no copy needed.
                            dma_eng.dma_start(out=m, in_=x[b, :, 0, :])
                        else:
                            xt = xpool.tile([C, w], F32, tag="xt")
                            dma_eng.dma_start(out=xt, in_=x[b, :, h, :])
                            nc.vector.tensor_add(out=m, in0=m, in1=xt)
                        if pending and h % 2 == 1:
                            attention_tail(*pending.pop(0))
                    pending.extend(
                        (b, k, ms[b][:, k * wt:(k + 1) * wt])
                        for k in range(nch)
                    )
                # Drain: after the final exp, dparts is complete, so the
                # denominator chain is emitted between the last tail's head
                # and rest — it runs in parallel with the weighted-sum tail
                # instead of serially after it.
                for item in pending[:-1]:
                    attention_tail(*item)
                lb, lk, lmt = pending[-1]
                let = tail_head(lb, lk, lmt)
                drow = accp.tile([1, bl], F32, tag="drow")
                nc.vector.reduce_sum(
                    out=drow, in_=dparts, axis=mybir.AxisListType.X)
                nc.scalar.mul(out=drow, in_=drow, mul=1.0 / H)
                nc.vector.reciprocal(out=drow, in_=drow)  # H/denom
                pdb = psmall.tile([C, bl], F32, tag="pdb")
                # (1/H) ones^T @ (H/denom) = 1/denom bcast to all partitions
                nc.tensor.matmul(pdb, sb_ones, drow, start=True, stop=True)
                tail_rest(lb, lk, lmt, let)

            rescol = accp.tile([C, bl], F32, tag="rescol")
            nc.vector.reduce_sum(out=rescol, in_=partials, axis=mybir.AxisListType.X)
            resn = accp.tile([C, bl], F32, tag="resn")
            nc.vector.tensor_mul(out=resn, in0=rescol, in1=pdb)

            # out[b, c] = resn[c, b]: transpose via matmul with identity.
            pt = psmall.tile([bl, C], F32, tag="small")
            nc.tensor.matmul(pt, resn, sb_ident, start=True, stop=True)
            out_sb = accp.tile([bl, C], F32, tag="out_sb")
            nc.vector.tensor_copy(out=out_sb, in_=pt)
            nc.sync.dma_start(out=out[:, :], in_=out_sb)

    nc.compile()
    return nc


def make_in_maps(x, weight_W, weight_proj, bias, bl=BL, n_cores=N_CORES):
    x = np.ascontiguousarray(np.asarray(x, dtype=np.float32))
    wpack = np.zeros((C, 386), dtype=np.float32)
    wpack[:, 0:C] = np.asarray(weight_W, dtype=np.float32) / np.float32(H)
    wpack[:, C:2 * C] = np.eye(C, dtype=np.float32)
    wpack[:, 2 * C:2 * C + 1] = np.asarray(weight_proj, dtype=np.float32)
    wpack[:, 2 * C + 1:2 * C + 2] = np.asarray(bias, dtype=np.float32)
    wpack[0, 2 * C + 2:3 * C + 2] = 1.0 / np.float32(H)
    return [
        {
            "x": np.ascontiguousarray(x[i * bl:(i + 1) * bl]),
            "wpack": wpack,
        }
        for i in range(n_cores)
    ]


_NC_CACHE = {}


def kernel(x, weight_W, weight_proj, bias, **run_kwargs):
    if "nc" not in _NC_CACHE:
        _NC_CACHE["nc"] = build_bass()
    nc = _NC_CACHE["nc"]
    in_maps = make_in_maps(x, weight_W, weight_proj, bias)
    res = None
    for attempt in range(3):
        try:
            res = run_bass_kernel_spmd(
                nc, in_maps, core_ids=list(range(N_CORES)), **run_kwargs)
            break
        except Exception:
            # Transient NRT/device hiccups recover on retry; re-raise if not.
            if attempt == 2:
                raise
    out = np.concatenate([r["out"] for r in res.results], axis=0)
    if run_kwargs:
        kernel.last_results = res
    return out
